# revision 21
# baseline (speedup 1.0000x reference)
# MoE FFN (top-2 of 8 experts) Trainium2 kernel.
#
# Strategy (expert-parallel, per sharding hint):
#   Phase A (8 cores):  router, token-sharded — exact-fp32 logits via PE,
#                       softmax/top-2/combine weights on device; per-shard
#                       aux-loss partial stats.
#   Host:               builds per-expert token gather lists from the device-
#                       computed combine weights (data movement only, no math),
#                       stages gathered/blocked operands per core.
#   Phase B (8 cores):  core e runs expert e's FFN over its gathered tokens:
#                       h = gelu(x @ w1.T + b1); y = (h @ w2.T + b2) * combine.
#                       MM1 in float32r (full-rate fp32), MM2 in bf16.
#                       Also reduces the aux-loss stats to the scalar loss.
#   Host:               routes each token's two expert contributions to its
#                       owner core (gather only).
#   Phase C (8 cores):  out = contribution_a + contribution_b (device add),
#                       token-sharded output.
import numpy as np

D_MODEL, D_FF, N_EXPERTS, TOP_K = 1024, 4096, 8, 2
B_DIM, S_DIM = 4, 2048
T = B_DIM * S_DIM          # 8192 tokens
NCORES = 8
TSH = T // NCORES          # tokens per phase-A shard
DEFAULT_CAP = 2304         # per-expert token capacity (multiple of 256)

_cache = {}
LAST_EXEC_NS = {}


def _run(nc, in_maps, core_ids, label):
    import os
    from concourse import bass_utils
    trace = bool(os.environ.get("MOE_TRACE"))
    res = bass_utils.run_bass_kernel_spmd(
        nc, in_maps, core_ids=core_ids, trace=trace
    )
    if trace:
        LAST_EXEC_NS[label] = res.exec_time_ns
    return res.results


def _mybir():
    import concourse.mybir as mybir
    return mybir


# ----------------------------------------------------------------- phase A
def _build_phase_a():
    import concourse.tile as tile
    from concourse import bacc
    mybir = _mybir()
    F32 = mybir.dt.float32
    AF = mybir.ActivationFunctionType
    OP = mybir.AluOpType
    AX = mybir.AxisListType
    NT = TSH // 128            # 8 token tiles per shard

    nc = bacc.Bacc("TRN2", target_bir_lowering=False, debug=False)
    xTs = nc.dram_tensor("xTs", [8, 128, TSH], F32, kind="ExternalInput")
    gwT = nc.dram_tensor("gwT", [8, 128, 8], F32, kind="ExternalInput")
    eye8 = nc.dram_tensor("eye8", [8, 8], F32, kind="ExternalInput")
    comb_o = nc.dram_tensor("combs", [TSH, 8], F32, kind="ExternalOutput")
    stat_o = nc.dram_tensor("stats", [8, 2], F32, kind="ExternalOutput")

    with tile.TileContext(nc) as tc:
        with (
            tc.tile_pool(name="const", bufs=1) as cpool,
            tc.tile_pool(name="xin", bufs=2) as xpool,
            tc.tile_pool(name="wide", bufs=1) as wpool,
            tc.tile_pool(name="small", bufs=2) as spool,
            tc.tile_pool(name="psA", bufs=2, space="PSUM") as ppa,
            tc.tile_pool(name="psB", bufs=2, space="PSUM") as ppb,
            tc.tile_pool(name="psum_stat", bufs=1, space="PSUM") as pstat,
        ):
            gw_sb = cpool.tile([128, 8, 8], F32)
            nc.sync.dma_start(gw_sb[:], gwT[:].transpose([1, 0, 2]))
            eye_sb = cpool.tile([8, 8], F32)
            nc.sync.dma_start(eye_sb[:], eye8[:])
            ones = cpool.tile([128, 1], F32)
            nc.vector.memset(ones[:], 1.0)

            # logits^T [8, TSH] via PE (exact fp32 — routing decisions must
            # match the fp32 reference at tiny top2/top3 gaps)
            lgT = wpool.tile([8, TSH], F32, tag="lgT")
            for tcn in range(TSH // 512):
                xch = xpool.tile([128, 8, 512], F32, tag="xch")
                nc.sync.dma_start(
                    xch[:],
                    xTs[:, :, tcn * 512:(tcn + 1) * 512].transpose([1, 0, 2]),
                )
                ps_t = ppa.tile([8, 512], F32, tag="pst")
                for di in range(8):
                    nc.tensor.matmul(
                        ps_t[:], gw_sb[:, di, :], xch[:, di, :],
                        start=(di == 0), stop=(di == 7),
                    )
                nc.vector.tensor_copy(lgT[:, tcn * 512:(tcn + 1) * 512], ps_t[:])

            # transpose to [128, NT, 8] (PE transpose via identity)
            lg = wpool.tile([128, NT, 8], F32, tag="lg")
            for tt in range(NT):
                trps = ppb.tile([128, 8], F32, tag="trps")
                nc.tensor.transpose(
                    trps[:], lgT[:, tt * 128:(tt + 1) * 128], eye_sb[:]
                )
                nc.vector.tensor_copy(lg[:, tt, :], trps[:])

            # wide softmax / top-2 over the expert axis (innermost, size 8)
            shp = [128, NT, 8]
            ex = wpool.tile(shp, F32, tag="ex")
            nc.scalar.activation(ex[:], lg[:], AF.Exp)
            s = wpool.tile([128, NT], F32, tag="s")
            nc.vector.tensor_reduce(s[:], ex[:], axis=AX.X, op=OP.add)
            r = wpool.tile([128, NT], F32, tag="r")
            nc.vector.reciprocal(r[:], s[:])
            p = wpool.tile(shp, F32, tag="p")
            nc.vector.tensor_tensor(
                p[:], ex[:], r[:].unsqueeze(2).broadcast_to(shp), OP.mult
            )
            m1 = wpool.tile([128, NT], F32, tag="m1")
            nc.vector.tensor_reduce(m1[:], ex[:], axis=AX.X, op=OP.max)
            mask1 = wpool.tile(shp, F32, tag="mask1")
            nc.vector.tensor_tensor(
                mask1[:], ex[:], m1[:].unsqueeze(2).broadcast_to(shp),
                OP.is_equal,
            )
            ex2 = wpool.tile(shp, F32, tag="ex2")
            nc.vector.scalar_tensor_tensor(
                ex2[:], mask1[:], -100.0, ex[:], op0=OP.mult, op1=OP.add
            )
            m2 = wpool.tile([128, NT], F32, tag="m2")
            nc.vector.tensor_reduce(m2[:], ex2[:], axis=AX.X, op=OP.max)
            mask2 = wpool.tile(shp, F32, tag="mask2")
            nc.vector.tensor_tensor(
                mask2[:], ex2[:], m2[:].unsqueeze(2).broadcast_to(shp),
                OP.is_equal,
            )
            s12 = wpool.tile([128, NT], F32, tag="s12")
            nc.vector.tensor_tensor(s12[:], m1[:], m2[:], OP.add)
            r12 = wpool.tile([128, NT], F32, tag="r12")
            nc.vector.reciprocal(r12[:], s12[:])
            c1 = wpool.tile([128, NT], F32, tag="c1")
            nc.vector.tensor_tensor(c1[:], m1[:], r12[:], OP.mult)
            c2 = wpool.tile([128, NT], F32, tag="c2")
            nc.vector.tensor_tensor(c2[:], m2[:], r12[:], OP.mult)
            t1 = wpool.tile(shp, F32, tag="t1")
            nc.vector.tensor_tensor(
                t1[:], mask1[:], c1[:].unsqueeze(2).broadcast_to(shp), OP.mult
            )
            t2 = wpool.tile(shp, F32, tag="t2")
            nc.vector.tensor_tensor(
                t2[:], mask2[:], c2[:].unsqueeze(2).broadcast_to(shp), OP.mult
            )
            comb = wpool.tile(shp, F32, tag="comb")
            nc.vector.tensor_tensor(comb[:], t1[:], t2[:], OP.add)
            # combine[t, e]  with t = tt*128 + partition
            nc.sync.dma_start(
                comb_o[:].rearrange("(a b) e -> b a e", b=128), comb[:]
            )

            # aux-loss partial stats for this shard: counts, prob sums
            msum = wpool.tile(shp, F32, tag="msum")
            nc.vector.tensor_tensor(msum[:], mask1[:], mask2[:], OP.add)
            cnt_red = spool.tile([128, 8], F32, tag="cnt")
            nc.vector.tensor_reduce(
                cnt_red[:], msum[:].transpose([0, 2, 1]), axis=AX.X, op=OP.add
            )
            p_red = spool.tile([128, 8], F32, tag="pred")
            nc.vector.tensor_reduce(
                p_red[:], p[:].transpose([0, 2, 1]), axis=AX.X, op=OP.add
            )
            ps_cnt = pstat.tile([8, 1], F32, tag="pscnt")
            nc.tensor.matmul(ps_cnt[:], cnt_red[:], ones[:], start=True, stop=True)
            ps_p = pstat.tile([8, 1], F32, tag="psp")
            nc.tensor.matmul(ps_p[:], p_red[:], ones[:], start=True, stop=True)
            stat_sb = spool.tile([8, 2], F32, tag="statsb")
            nc.vector.tensor_copy(stat_sb[:, 0:1], ps_cnt[:])
            nc.vector.tensor_copy(stat_sb[:, 1:2], ps_p[:])
            nc.sync.dma_start(stat_o[:], stat_sb[:])

    nc.compile()
    return nc


def _build_phase_b_aux(nc, tc, tile, mybir, stats, aux_o, spool):
    # DVE-only on a single partition; pool stays open for the whole kernel
    # (early-closed pools corrupt later allocations).
    # stats layout: [1, kind(2), expert(8), core(8)] on partition 0.
    F32 = mybir.dt.float32
    OP = mybir.AluOpType
    AX = mybir.AxisListType
    st_sb = spool.tile([1, 2, 8, 8], F32, tag="stsb")
    nc.gpsimd.dma_start(st_sb[:], stats[:])
    red = spool.tile([1, 2, 8], F32, tag="red")
    nc.vector.tensor_reduce(red[:], st_sb[:], axis=AX.X, op=OP.add)
    terms = spool.tile([1, 8], F32, tag="terms")
    nc.vector.tensor_tensor(terms[:], red[:, 0, :], red[:, 1, :], OP.mult)
    tot = spool.tile([1, 1], F32, tag="tot")
    nc.vector.tensor_reduce(tot[:], terms[:], axis=AX.X, op=OP.add)
    aux_sb = spool.tile([1, 1], F32, tag="auxsb")
    scale = float(N_EXPERTS) / (float(T) * TOP_K * float(T))
    nc.scalar.mul(aux_sb[:], tot[:], scale)
    nc.gpsimd.dma_start(aux_o[:], aux_sb[:])


# ----------------------------------------------------------------- phase B
def _build_phase_b(cap):
    import concourse.tile as tile
    from concourse import bacc
    mybir = _mybir()
    F32 = mybir.dt.float32
    F32R = mybir.dt.float32r
    BF16 = mybir.dt.bfloat16
    AF = mybir.ActivationFunctionType
    OP = mybir.AluOpType

    assert cap % 256 == 0
    # token groups of <=1280 (h for one group stays SBUF-resident);
    # each group is cut into matmul chunks of >=256 columns.
    groups = []
    g0 = 0
    ngroups = -(-cap // 1280)
    gsize = -(-cap // ngroups // 256) * 256
    while g0 < cap:
        gs = min(gsize, cap - g0)
        nch = -(-gs // 512)
        base = gs // nch // 128 * 128
        chunks = []
        o = 0
        for ci in range(nch):
            nk = base if ci < nch - 1 else gs - base * (nch - 1)
            chunks.append((o, nk))
            o += nk
        assert all(256 <= nk <= 512 for _, nk in chunks), chunks
        groups.append((g0, gs, chunks))
        g0 += gs

    nc = bacc.Bacc("TRN2", target_bir_lowering=False, debug=False)
    xg = nc.dram_tensor("xg", [8, 128, cap], F32R, kind="ExternalInput")
    w1 = nc.dram_tensor("w1b", [32, 8, 128, 128], F32R, kind="ExternalInput")
    w2 = nc.dram_tensor("w2b", [8, 128, 32, 128], F32, kind="ExternalInput")
    b1 = nc.dram_tensor("b1t", [128, 32], F32, kind="ExternalInput")
    b2 = nc.dram_tensor("b2t", [128, 8], F32, kind="ExternalInput")
    crep = nc.dram_tensor("crep", [128, cap], F32, kind="ExternalInput")
    stats = nc.dram_tensor("stats", [1, 2, 8, 8], F32, kind="ExternalInput")
    y = nc.dram_tensor("y", [8, 128, cap], F32, kind="ExternalOutput")
    aux_o = nc.dram_tensor("aux", [1, 1], F32, kind="ExternalOutput")

    with tile.TileContext(nc) as tc:
        with (
            tc.tile_pool(name="const", bufs=1) as cpool,
            tc.tile_pool(name="small", bufs=1) as spool,
            tc.tile_pool(name="dram", bufs=1, space="DRAM") as dpool,
        ):
            b1_sb = cpool.tile([128, 32], F32)
            nc.sync.dma_start(b1_sb[:], b1[:])
            b2_sb = cpool.tile([128, 8], F32)
            nc.sync.dma_start(b2_sb[:], b2[:])
            c_sb = cpool.tile([128, cap], F32)
            nc.gpsimd.dma_start(c_sb[:], crep[:])

            # aux loss from the phase-A partial stats (tiny, replicated)
            _build_phase_b_aux(nc, tc, tile, mybir, stats, aux_o, spool)
            # one-time cast of w2 to bf16 (DRAM->DRAM via SBUF)
            w2bf_d = dpool.tile([8, 128, 32, 128], BF16)
            with (
                tc.tile_pool(name="w2stg", bufs=2) as stgpool,
                tc.tile_pool(name="w2c", bufs=2) as w2cpool,
            ):
                for di in range(8):
                    stg = stgpool.tile([128, 32, 128], F32, tag="stg")
                    nc.gpsimd.dma_start(stg[:], w2[di])
                    w2c = w2cpool.tile([128, 32, 128], BF16, tag="w2c")
                    nc.vector.tensor_copy(w2c[:], stg[:])
                    nc.gpsimd.dma_start(w2bf_d[di], w2c[:])

            with (
                tc.tile_pool(name="xg", bufs=2) as xpool,
                tc.tile_pool(name="h", bufs=1) as hpool,
                tc.tile_pool(name="w1", bufs=3) as w1pool,
                tc.tile_pool(name="w2s", bufs=2) as w2spool,
                tc.tile_pool(name="yout", bufs=3) as ypool,
                tc.tile_pool(name="psum", bufs=6, space="PSUM") as pp,
            ):
                for (g0, gs, chunks) in groups:
                    xgs = xpool.tile([128, 8, gs], F32R, tag="xgs")
                    nc.sync.dma_start(
                        xgs[:], xg[:, :, g0:g0 + gs].transpose([1, 0, 2])
                    )
                    h_sb = hpool.tile([128, 32, gs], BF16, tag="h")
                    # ---- MM1: h = gelu(x @ w1.T + b1)  (float32r)
                    for fi in range(32):
                        w1t = w1pool.tile([128, 8, 128], F32R, tag="w1t")
                        nc.sync.dma_start(w1t[:], w1[fi].transpose([1, 0, 2]))
                        pss = []
                        for ci, (o, nk) in enumerate(chunks):
                            pst = pp.tile([128, 512], F32, tag="ps",
                                          name=f"ps{ci}")
                            pss.append(pst)
                        for di in range(8):
                            for ci, (o, nk) in enumerate(chunks):
                                nc.tensor.matmul(
                                    pss[ci][:, 0:nk],
                                    w1t[:, di, :],
                                    xgs[:, di, o:o + nk],
                                    start=(di == 0),
                                    stop=(di == 7),
                                )
                        for ci, (o, nk) in enumerate(chunks):
                            nc.scalar.activation(
                                h_sb[:, fi, o:o + nk], pss[ci][:, 0:nk],
                                AF.Gelu, bias=b1_sb[:, fi:fi + 1],
                            )
                    # ---- MM2: y = (h @ w2.T + b2) * combine   (bf16)
                    for di in range(8):
                        w2t = w2spool.tile([128, 32, 128], BF16, tag="w2t")
                        nc.sync.dma_start(w2t[:], w2bf_d[di])
                        for (o, nk) in chunks:
                            psy = pp.tile([128, 512], F32, tag="ps")
                            for fj in range(32):
                                nc.tensor.matmul(
                                    psy[:, 0:nk],
                                    w2t[:, fj, :],
                                    h_sb[:, fj, o:o + nk],
                                    start=(fj == 0),
                                    stop=(fj == 31),
                                )
                            yt = ypool.tile([128, 512], F32, tag="yt")
                            nc.vector.scalar_tensor_tensor(
                                yt[:, 0:nk], psy[:, 0:nk], b2_sb[:, di:di + 1],
                                c_sb[:, g0 + o:g0 + o + nk],
                                op0=OP.add, op1=OP.mult,
                            )
                            nc.sync.dma_start(
                                y[di, :, g0 + o:g0 + o + nk], yt[:, 0:nk]
                            )

    nc.compile()
    return nc


# ----------------------------------------------------------------- phase C
def _build_phase_c():
    import concourse.tile as tile
    from concourse import bacc
    mybir = _mybir()
    F32 = mybir.dt.float32
    OP = mybir.AluOpType

    nc = bacc.Bacc("TRN2", target_bir_lowering=False, debug=False)
    a_in = nc.dram_tensor("ca", [8, 128, 1024], F32, kind="ExternalInput")
    b_in = nc.dram_tensor("cb", [8, 128, 1024], F32, kind="ExternalInput")
    o_out = nc.dram_tensor("co", [8, 128, 1024], F32, kind="ExternalOutput")

    with tile.TileContext(nc) as tc:
        with (
            tc.tile_pool(name="io", bufs=2) as iop,
        ):
            for half in range(2):
                sl = slice(half * 4, half * 4 + 4)
                at = iop.tile([128, 4, 1024], F32, tag="a")
                nc.sync.dma_start(at[:], a_in[sl].transpose([1, 0, 2]))
                bt = iop.tile([128, 4, 1024], F32, tag="b")
                nc.sync.dma_start(bt[:], b_in[sl].transpose([1, 0, 2]))
                ot = iop.tile([128, 4, 1024], F32, tag="o")
                nc.vector.tensor_tensor(ot[:], at[:], bt[:], OP.add)
                nc.sync.dma_start(o_out[sl].transpose([1, 0, 2]), ot[:])

    nc.compile()
    return nc


def _get(phase, *args):
    key = (phase,) + args
    if key not in _cache:
        if phase == "a":
            _cache[key] = _build_phase_a()
        elif phase == "b":
            _cache[key] = _build_phase_b(*args)
        else:
            _cache[key] = _build_phase_c()
    return _cache[key]


# ----------------------------------------------------------------- driver
def kernel(x, gate_w, w1, b1, w2, b2):
    x = np.ascontiguousarray(np.asarray(x, np.float32))
    gate_w = np.ascontiguousarray(np.asarray(gate_w, np.float32))
    w1 = np.ascontiguousarray(np.asarray(w1, np.float32))
    b1 = np.ascontiguousarray(np.asarray(b1, np.float32))
    w2 = np.ascontiguousarray(np.asarray(w2, np.float32))
    b2 = np.ascontiguousarray(np.asarray(b2, np.float32))

    x_flat = x.reshape(T, D_MODEL)

    # ---- phase A: routing on device (8 cores, token-sharded)
    xT = x_flat.T.reshape(8, 128, T)
    gwT = np.ascontiguousarray(gate_w.T.reshape(8, 128, 8))
    eye8 = np.eye(8, dtype=np.float32)
    nc_a = _get("a")
    in_maps_a = [
        {
            "xTs": np.ascontiguousarray(xT[:, :, r * TSH:(r + 1) * TSH]),
            "gwT": gwT,
            "eye8": eye8,
        }
        for r in range(NCORES)
    ]
    res_a = _run(nc_a, in_maps_a, list(range(NCORES)), "a")
    combine = np.concatenate([res_a[r]["combs"] for r in range(NCORES)])
    stats_v = np.stack([res_a[r]["stats"] for r in range(NCORES)])  # (core,e,kind)
    stats_in = np.ascontiguousarray(
        stats_v.transpose(2, 1, 0)[None].astype(np.float32)  # (1,kind,e,core)
    )

    # ---- host: build gather lists (indexing only)
    nz = combine > 0
    ids = [np.nonzero(nz[:, e])[0] for e in range(N_EXPERTS)]
    maxcnt = max(len(i) for i in ids)
    cap = DEFAULT_CAP
    while cap < maxcnt:
        cap += 256

    nc_b = _get("b", cap)
    in_maps = []
    for e in range(N_EXPERTS):
        n = len(ids[e])
        xe = np.zeros((cap, D_MODEL), np.float32)
        xe[:n] = x_flat[ids[e]]
        xg_blk = np.ascontiguousarray(xe.T.reshape(8, 128, cap))
        w1_blk = np.ascontiguousarray(
            w1[e].reshape(32, 128, 8, 128).transpose(0, 2, 3, 1)
        )
        w2_blk = np.ascontiguousarray(
            w2[e].reshape(8, 128, 32, 128).transpose(0, 3, 2, 1)
        )
        b1_t = np.ascontiguousarray(b1[e].reshape(32, 128).T)
        b2_t = np.ascontiguousarray(b2[e].reshape(8, 128).T)
        ce = np.zeros((cap,), np.float32)
        ce[:n] = combine[ids[e], e]
        c_rep = np.ascontiguousarray(np.broadcast_to(ce, (128, cap)))
        in_maps.append({
            "xg": xg_blk, "w1b": w1_blk, "w2b": w2_blk,
            "b1t": b1_t, "b2t": b2_t, "crep": c_rep, "stats": stats_in,
        })
    res_b = _run(nc_b, in_maps, list(range(NCORES)), "b")
    aux = np.float32(res_b[0]["aux"].reshape(()))
    # y blocks -> (cap, D) scaled contributions per expert
    ye_all = np.stack([
        res_b[e]["y"].transpose(2, 0, 1).reshape(cap, D_MODEL)
        for e in range(N_EXPERTS)
    ])

    # ---- host: route contributions to token owners (gather only)
    tcol, ecol = np.nonzero(nz)          # row-major: token-major pairs
    e1 = ecol[0::2]
    e2 = ecol[1::2]
    posmap = np.zeros((T, N_EXPERTS), np.int64)
    for e in range(N_EXPERTS):
        posmap[ids[e], e] = np.arange(len(ids[e]))
    tok = np.arange(T)
    p1 = posmap[tok, e1]
    p2 = posmap[tok, e2]
    a_all = ye_all[e1, p1]               # (T, D) gather
    b_all = ye_all[e2, p2]

    nc_c = _get("c")
    in_maps_c = [
        {
            "ca": np.ascontiguousarray(
                a_all[r * 1024:(r + 1) * 1024].reshape(8, 128, D_MODEL)
            ),
            "cb": np.ascontiguousarray(
                b_all[r * 1024:(r + 1) * 1024].reshape(8, 128, D_MODEL)
            ),
        }
        for r in range(NCORES)
    ]
    res_c = _run(nc_c, in_maps_c, list(range(NCORES)), "c")
    out = np.concatenate(
        [res_c[r]["co"].reshape(1024, D_MODEL) for r in range(NCORES)]
    ).reshape(B_DIM, S_DIM, D_MODEL)
    return out, aux


# revision 23
# speedup vs baseline: 1.1236x; 1.1236x over previous
# MoE FFN (top-2 of 8 experts) Trainium2 kernel.
#
# Strategy (expert-parallel, per sharding hint):
#   Phase A (8 cores):  router, token-sharded — exact-fp32 logits via PE,
#                       softmax/top-2/combine weights on device; per-shard
#                       aux-loss partial stats.
#   Host:               builds per-expert token gather lists from the device-
#                       computed combine weights (data movement only, no math),
#                       stages gathered/blocked operands per core.
#   Phase B (8 cores):  core e runs expert e's FFN over its gathered tokens:
#                       h = gelu(x @ w1.T + b1); y = (h @ w2.T + b2) * combine.
#                       MM1 in float32r (full-rate fp32), MM2 in bf16.
#                       Also reduces the aux-loss stats to the scalar loss.
#   Host:               routes each token's two expert contributions to its
#                       owner core (gather only).
#   Phase C (8 cores):  out = contribution_a + contribution_b (device add),
#                       token-sharded output.
import numpy as np

D_MODEL, D_FF, N_EXPERTS, TOP_K = 1024, 4096, 8, 2
B_DIM, S_DIM = 4, 2048
T = B_DIM * S_DIM          # 8192 tokens
NCORES = 8
TSH = T // NCORES          # tokens per phase-A shard
DEFAULT_CAP = 2304         # per-expert token capacity (multiple of 256)

_cache = {}
LAST_EXEC_NS = {}


def _run(nc, in_maps, core_ids, label):
    import os
    from concourse import bass_utils
    trace = bool(os.environ.get("MOE_TRACE"))
    res = bass_utils.run_bass_kernel_spmd(
        nc, in_maps, core_ids=core_ids, trace=trace
    )
    if trace:
        LAST_EXEC_NS[label] = res.exec_time_ns
    return res.results


def _mybir():
    import concourse.mybir as mybir
    return mybir


# ----------------------------------------------------------------- phase A
def _build_phase_a():
    import concourse.tile as tile
    from concourse import bacc
    mybir = _mybir()
    F32 = mybir.dt.float32
    AF = mybir.ActivationFunctionType
    OP = mybir.AluOpType
    AX = mybir.AxisListType
    NT = TSH // 128            # 8 token tiles per shard

    nc = bacc.Bacc("TRN2", target_bir_lowering=False, debug=False)
    xTs = nc.dram_tensor("xTs", [8, 128, TSH], F32, kind="ExternalInput")
    gwT = nc.dram_tensor("gwT", [8, 128, 8], F32, kind="ExternalInput")
    eye8 = nc.dram_tensor("eye8", [8, 8], F32, kind="ExternalInput")
    comb_o = nc.dram_tensor("combs", [TSH, 8], F32, kind="ExternalOutput")
    stat_o = nc.dram_tensor("stats", [8, 2], F32, kind="ExternalOutput")

    with tile.TileContext(nc) as tc:
        with (
            tc.tile_pool(name="const", bufs=1) as cpool,
            tc.tile_pool(name="xin", bufs=2) as xpool,
            tc.tile_pool(name="wide", bufs=1) as wpool,
            tc.tile_pool(name="small", bufs=2) as spool,
            tc.tile_pool(name="psA", bufs=2, space="PSUM") as ppa,
            tc.tile_pool(name="psB", bufs=2, space="PSUM") as ppb,
            tc.tile_pool(name="psum_stat", bufs=1, space="PSUM") as pstat,
        ):
            gw_sb = cpool.tile([128, 8, 8], F32)
            nc.sync.dma_start(gw_sb[:], gwT[:].transpose([1, 0, 2]))
            eye_sb = cpool.tile([8, 8], F32)
            nc.sync.dma_start(eye_sb[:], eye8[:])
            ones = cpool.tile([128, 1], F32)
            nc.vector.memset(ones[:], 1.0)

            # logits^T [8, TSH] via PE (exact fp32 — routing decisions must
            # match the fp32 reference at tiny top2/top3 gaps)
            lgT = wpool.tile([8, TSH], F32, tag="lgT")
            for tcn in range(TSH // 512):
                xch = xpool.tile([128, 8, 512], F32, tag="xch")
                nc.sync.dma_start(
                    xch[:],
                    xTs[:, :, tcn * 512:(tcn + 1) * 512].transpose([1, 0, 2]),
                )
                ps_t = ppa.tile([8, 512], F32, tag="pst")
                for di in range(8):
                    nc.tensor.matmul(
                        ps_t[:], gw_sb[:, di, :], xch[:, di, :],
                        start=(di == 0), stop=(di == 7),
                    )
                nc.vector.tensor_copy(lgT[:, tcn * 512:(tcn + 1) * 512], ps_t[:])

            # transpose to [128, NT, 8] (PE transpose via identity)
            lg = wpool.tile([128, NT, 8], F32, tag="lg")
            for tt in range(NT):
                trps = ppb.tile([128, 8], F32, tag="trps")
                nc.tensor.transpose(
                    trps[:], lgT[:, tt * 128:(tt + 1) * 128], eye_sb[:]
                )
                nc.vector.tensor_copy(lg[:, tt, :], trps[:])

            # wide softmax / top-2 over the expert axis (innermost, size 8)
            shp = [128, NT, 8]
            ex = wpool.tile(shp, F32, tag="ex")
            nc.scalar.activation(ex[:], lg[:], AF.Exp)
            s = wpool.tile([128, NT], F32, tag="s")
            nc.vector.tensor_reduce(s[:], ex[:], axis=AX.X, op=OP.add)
            r = wpool.tile([128, NT], F32, tag="r")
            nc.vector.reciprocal(r[:], s[:])
            p = wpool.tile(shp, F32, tag="p")
            nc.vector.tensor_tensor(
                p[:], ex[:], r[:].unsqueeze(2).broadcast_to(shp), OP.mult
            )
            m1 = wpool.tile([128, NT], F32, tag="m1")
            nc.vector.tensor_reduce(m1[:], ex[:], axis=AX.X, op=OP.max)
            mask1 = wpool.tile(shp, F32, tag="mask1")
            nc.vector.tensor_tensor(
                mask1[:], ex[:], m1[:].unsqueeze(2).broadcast_to(shp),
                OP.is_equal,
            )
            ex2 = wpool.tile(shp, F32, tag="ex2")
            nc.vector.scalar_tensor_tensor(
                ex2[:], mask1[:], -100.0, ex[:], op0=OP.mult, op1=OP.add
            )
            m2 = wpool.tile([128, NT], F32, tag="m2")
            nc.vector.tensor_reduce(m2[:], ex2[:], axis=AX.X, op=OP.max)
            mask2 = wpool.tile(shp, F32, tag="mask2")
            nc.vector.tensor_tensor(
                mask2[:], ex2[:], m2[:].unsqueeze(2).broadcast_to(shp),
                OP.is_equal,
            )
            s12 = wpool.tile([128, NT], F32, tag="s12")
            nc.vector.tensor_tensor(s12[:], m1[:], m2[:], OP.add)
            r12 = wpool.tile([128, NT], F32, tag="r12")
            nc.vector.reciprocal(r12[:], s12[:])
            c1 = wpool.tile([128, NT], F32, tag="c1")
            nc.vector.tensor_tensor(c1[:], m1[:], r12[:], OP.mult)
            c2 = wpool.tile([128, NT], F32, tag="c2")
            nc.vector.tensor_tensor(c2[:], m2[:], r12[:], OP.mult)
            t1 = wpool.tile(shp, F32, tag="t1")
            nc.vector.tensor_tensor(
                t1[:], mask1[:], c1[:].unsqueeze(2).broadcast_to(shp), OP.mult
            )
            t2 = wpool.tile(shp, F32, tag="t2")
            nc.vector.tensor_tensor(
                t2[:], mask2[:], c2[:].unsqueeze(2).broadcast_to(shp), OP.mult
            )
            comb = wpool.tile(shp, F32, tag="comb")
            nc.vector.tensor_tensor(comb[:], t1[:], t2[:], OP.add)
            # combine[t, e]  with t = tt*128 + partition
            nc.sync.dma_start(
                comb_o[:].rearrange("(a b) e -> b a e", b=128), comb[:]
            )

            # aux-loss partial stats for this shard: counts, prob sums
            msum = wpool.tile(shp, F32, tag="msum")
            nc.vector.tensor_tensor(msum[:], mask1[:], mask2[:], OP.add)
            cnt_red = spool.tile([128, 8], F32, tag="cnt")
            nc.vector.tensor_reduce(
                cnt_red[:], msum[:].transpose([0, 2, 1]), axis=AX.X, op=OP.add
            )
            p_red = spool.tile([128, 8], F32, tag="pred")
            nc.vector.tensor_reduce(
                p_red[:], p[:].transpose([0, 2, 1]), axis=AX.X, op=OP.add
            )
            ps_cnt = pstat.tile([8, 1], F32, tag="pscnt")
            nc.tensor.matmul(ps_cnt[:], cnt_red[:], ones[:], start=True, stop=True)
            ps_p = pstat.tile([8, 1], F32, tag="psp")
            nc.tensor.matmul(ps_p[:], p_red[:], ones[:], start=True, stop=True)
            stat_sb = spool.tile([8, 2], F32, tag="statsb")
            nc.vector.tensor_copy(stat_sb[:, 0:1], ps_cnt[:])
            nc.vector.tensor_copy(stat_sb[:, 1:2], ps_p[:])
            nc.sync.dma_start(stat_o[:], stat_sb[:])

    nc.compile()
    return nc


def _build_phase_b_aux(nc, tc, tile, mybir, stats, aux_o, spool):
    # DVE-only on a single partition; pool stays open for the whole kernel
    # (early-closed pools corrupt later allocations).
    # stats layout: [1, kind(2), expert(8), core(8)] on partition 0.
    F32 = mybir.dt.float32
    OP = mybir.AluOpType
    AX = mybir.AxisListType
    st_sb = spool.tile([1, 2, 8, 8], F32, tag="stsb")
    nc.gpsimd.dma_start(st_sb[:], stats[:])
    red = spool.tile([1, 2, 8], F32, tag="red")
    nc.vector.tensor_reduce(red[:], st_sb[:], axis=AX.X, op=OP.add)
    terms = spool.tile([1, 8], F32, tag="terms")
    nc.vector.tensor_tensor(terms[:], red[:, 0, :], red[:, 1, :], OP.mult)
    tot = spool.tile([1, 1], F32, tag="tot")
    nc.vector.tensor_reduce(tot[:], terms[:], axis=AX.X, op=OP.add)
    aux_sb = spool.tile([1, 1], F32, tag="auxsb")
    scale = float(N_EXPERTS) / (float(T) * TOP_K * float(T))
    nc.scalar.mul(aux_sb[:], tot[:], scale)
    nc.gpsimd.dma_start(aux_o[:], aux_sb[:])


# ----------------------------------------------------------------- phase B
def _build_phase_b(cap):
    import concourse.tile as tile
    from concourse import bacc
    mybir = _mybir()
    F32 = mybir.dt.float32
    F32R = mybir.dt.float32r
    BF16 = mybir.dt.bfloat16
    AF = mybir.ActivationFunctionType
    OP = mybir.AluOpType

    assert cap % 256 == 0
    # token groups of <=1280 (h for one group stays SBUF-resident);
    # each group is cut into matmul chunks of >=256 columns.
    groups = []
    g0 = 0
    ngroups = -(-cap // 1280)
    gsize = -(-cap // ngroups // 256) * 256
    while g0 < cap:
        gs = min(gsize, cap - g0)
        nch = -(-gs // 512)
        base = gs // nch // 128 * 128
        chunks = []
        o = 0
        for ci in range(nch):
            nk = base if ci < nch - 1 else gs - base * (nch - 1)
            chunks.append((o, nk))
            o += nk
        assert all(256 <= nk <= 512 for _, nk in chunks), chunks
        groups.append((g0, gs, chunks))
        g0 += gs

    nc = bacc.Bacc("TRN2", target_bir_lowering=False, debug=False)
    xg = nc.dram_tensor("xg", [8, 128, cap], F32R, kind="ExternalInput")
    w1 = nc.dram_tensor("w1b", [32, 8, 128, 128], F32R, kind="ExternalInput")
    w2 = nc.dram_tensor("w2b", [8, 128, 32, 128], F32, kind="ExternalInput")
    b1 = nc.dram_tensor("b1t", [128, 32], F32, kind="ExternalInput")
    b2 = nc.dram_tensor("b2t", [128, 8], F32, kind="ExternalInput")
    crep = nc.dram_tensor("crep", [128, cap], F32, kind="ExternalInput")
    stats = nc.dram_tensor("stats", [1, 2, 8, 8], F32, kind="ExternalInput")
    y = nc.dram_tensor("y", [8, 128, cap], F32, kind="ExternalOutput")
    aux_o = nc.dram_tensor("aux", [1, 1], F32, kind="ExternalOutput")

    with tile.TileContext(nc) as tc:
        with (
            tc.tile_pool(name="const", bufs=1) as cpool,
            tc.tile_pool(name="small", bufs=1) as spool,
            tc.tile_pool(name="dram", bufs=1, space="DRAM") as dpool,
        ):
            b1_sb = cpool.tile([128, 32], F32)
            nc.sync.dma_start(b1_sb[:], b1[:])
            b2_sb = cpool.tile([128, 8], F32)
            nc.sync.dma_start(b2_sb[:], b2[:])
            c_sb = cpool.tile([128, cap], F32)
            nc.gpsimd.dma_start(c_sb[:], crep[:])

            # aux loss from the phase-A partial stats (tiny, replicated)
            _build_phase_b_aux(nc, tc, tile, mybir, stats, aux_o, spool)
            # w2 -> bf16 cast (DRAM->DRAM via SBUF), interleaved into the
            # first group's MM1 so its DMA traffic stays off the critical
            # path of the first xg/w1 loads.
            w2bf_d = dpool.tile([8, 128, 32, 128], BF16)

            def w2_cast_step(step):
                di, hf = step // 2, (step % 2) * 16
                stg = stgpool.tile([128, 16, 128], F32, tag="stg")
                nc.gpsimd.dma_start(stg[:], w2[di, :, hf:hf + 16, :])
                w2c = w2cpool.tile([128, 16, 128], BF16, tag="w2c")
                nc.vector.tensor_copy(w2c[:], stg[:])
                nc.gpsimd.dma_start(w2bf_d[di, :, hf:hf + 16, :], w2c[:])

            with (
                tc.tile_pool(name="w2stg", bufs=1) as stgpool,
                tc.tile_pool(name="w2c", bufs=1) as w2cpool,
                tc.tile_pool(name="xg", bufs=1) as xpool,
                tc.tile_pool(name="h", bufs=1) as hpool,
                tc.tile_pool(name="w1", bufs=3) as w1pool,
                tc.tile_pool(name="w2s", bufs=2) as w2spool,
                tc.tile_pool(name="yout", bufs=3) as ypool,
                tc.tile_pool(name="psum", bufs=6, space="PSUM") as pp,
            ):
                for gi, (g0, gs, chunks) in enumerate(groups):
                    xgs = xpool.tile([128, 8, gs], F32R, tag="xgs")
                    nc.sync.dma_start(
                        xgs[:], xg[:, :, g0:g0 + gs].transpose([1, 0, 2])
                    )
                    h_sb = hpool.tile([128, 32, gs], BF16, tag="h")
                    # ---- MM1: h = gelu(x @ w1.T + b1)  (float32r)
                    for fi in range(32):
                        w1t = w1pool.tile([128, 8, 128], F32R, tag="w1t")
                        nc.sync.dma_start(w1t[:], w1[fi].transpose([1, 0, 2]))
                        pss = []
                        for ci, (o, nk) in enumerate(chunks):
                            pst = pp.tile([128, 512], F32, tag="ps",
                                          name=f"ps{ci}")
                            pss.append(pst)
                        for di in range(8):
                            for ci, (o, nk) in enumerate(chunks):
                                nc.tensor.matmul(
                                    pss[ci][:, 0:nk],
                                    w1t[:, di, :],
                                    xgs[:, di, o:o + nk],
                                    start=(di == 0),
                                    stop=(di == 7),
                                )
                        for ci, (o, nk) in enumerate(chunks):
                            nc.scalar.activation(
                                h_sb[:, fi, o:o + nk], pss[ci][:, 0:nk],
                                AF.Gelu, bias=b1_sb[:, fi:fi + 1],
                            )
                        if gi == 0 and fi % 2 == 1:
                            w2_cast_step(fi // 2)
                    # ---- MM2: y = (h @ w2.T + b2) * combine   (bf16)
                    for di in range(8):
                        w2t = w2spool.tile([128, 32, 128], BF16, tag="w2t")
                        nc.sync.dma_start(w2t[:], w2bf_d[di])
                        for (o, nk) in chunks:
                            psy = pp.tile([128, 512], F32, tag="ps")
                            for fj in range(32):
                                nc.tensor.matmul(
                                    psy[:, 0:nk],
                                    w2t[:, fj, :],
                                    h_sb[:, fj, o:o + nk],
                                    start=(fj == 0),
                                    stop=(fj == 31),
                                )
                            yt = ypool.tile([128, 512], F32, tag="yt")
                            nc.vector.scalar_tensor_tensor(
                                yt[:, 0:nk], psy[:, 0:nk], b2_sb[:, di:di + 1],
                                c_sb[:, g0 + o:g0 + o + nk],
                                op0=OP.add, op1=OP.mult,
                            )
                            nc.sync.dma_start(
                                y[di, :, g0 + o:g0 + o + nk], yt[:, 0:nk]
                            )

    nc.compile()
    return nc


# ----------------------------------------------------------------- phase C
def _build_phase_c():
    import concourse.tile as tile
    from concourse import bacc
    mybir = _mybir()
    F32 = mybir.dt.float32
    OP = mybir.AluOpType

    nc = bacc.Bacc("TRN2", target_bir_lowering=False, debug=False)
    a_in = nc.dram_tensor("ca", [8, 128, 1024], F32, kind="ExternalInput")
    b_in = nc.dram_tensor("cb", [8, 128, 1024], F32, kind="ExternalInput")
    o_out = nc.dram_tensor("co", [8, 128, 1024], F32, kind="ExternalOutput")

    with tile.TileContext(nc) as tc:
        with (
            tc.tile_pool(name="io", bufs=2) as iop,
        ):
            for half in range(2):
                sl = slice(half * 4, half * 4 + 4)
                at = iop.tile([128, 4, 1024], F32, tag="a")
                nc.sync.dma_start(at[:], a_in[sl].transpose([1, 0, 2]))
                bt = iop.tile([128, 4, 1024], F32, tag="b")
                nc.sync.dma_start(bt[:], b_in[sl].transpose([1, 0, 2]))
                ot = iop.tile([128, 4, 1024], F32, tag="o")
                nc.vector.tensor_tensor(ot[:], at[:], bt[:], OP.add)
                nc.sync.dma_start(o_out[sl].transpose([1, 0, 2]), ot[:])

    nc.compile()
    return nc


def _get(phase, *args):
    key = (phase,) + args
    if key not in _cache:
        if phase == "a":
            _cache[key] = _build_phase_a()
        elif phase == "b":
            _cache[key] = _build_phase_b(*args)
        else:
            _cache[key] = _build_phase_c()
    return _cache[key]


# ----------------------------------------------------------------- driver
def kernel(x, gate_w, w1, b1, w2, b2):
    x = np.ascontiguousarray(np.asarray(x, np.float32))
    gate_w = np.ascontiguousarray(np.asarray(gate_w, np.float32))
    w1 = np.ascontiguousarray(np.asarray(w1, np.float32))
    b1 = np.ascontiguousarray(np.asarray(b1, np.float32))
    w2 = np.ascontiguousarray(np.asarray(w2, np.float32))
    b2 = np.ascontiguousarray(np.asarray(b2, np.float32))

    x_flat = x.reshape(T, D_MODEL)

    # ---- phase A: routing on device (8 cores, token-sharded)
    xT = x_flat.T.reshape(8, 128, T)
    gwT = np.ascontiguousarray(gate_w.T.reshape(8, 128, 8))
    eye8 = np.eye(8, dtype=np.float32)
    nc_a = _get("a")
    in_maps_a = [
        {
            "xTs": np.ascontiguousarray(xT[:, :, r * TSH:(r + 1) * TSH]),
            "gwT": gwT,
            "eye8": eye8,
        }
        for r in range(NCORES)
    ]
    res_a = _run(nc_a, in_maps_a, list(range(NCORES)), "a")
    combine = np.concatenate([res_a[r]["combs"] for r in range(NCORES)])
    stats_v = np.stack([res_a[r]["stats"] for r in range(NCORES)])  # (core,e,kind)
    stats_in = np.ascontiguousarray(
        stats_v.transpose(2, 1, 0)[None].astype(np.float32)  # (1,kind,e,core)
    )

    # ---- host: build gather lists (indexing only)
    nz = combine > 0
    ids = [np.nonzero(nz[:, e])[0] for e in range(N_EXPERTS)]
    maxcnt = max(len(i) for i in ids)
    cap = DEFAULT_CAP
    while cap < maxcnt:
        cap += 256

    nc_b = _get("b", cap)
    in_maps = []
    for e in range(N_EXPERTS):
        n = len(ids[e])
        xe = np.zeros((cap, D_MODEL), np.float32)
        xe[:n] = x_flat[ids[e]]
        xg_blk = np.ascontiguousarray(xe.T.reshape(8, 128, cap))
        w1_blk = np.ascontiguousarray(
            w1[e].reshape(32, 128, 8, 128).transpose(0, 2, 3, 1)
        )
        w2_blk = np.ascontiguousarray(
            w2[e].reshape(8, 128, 32, 128).transpose(0, 3, 2, 1)
        )
        b1_t = np.ascontiguousarray(b1[e].reshape(32, 128).T)
        b2_t = np.ascontiguousarray(b2[e].reshape(8, 128).T)
        ce = np.zeros((cap,), np.float32)
        ce[:n] = combine[ids[e], e]
        c_rep = np.ascontiguousarray(np.broadcast_to(ce, (128, cap)))
        in_maps.append({
            "xg": xg_blk, "w1b": w1_blk, "w2b": w2_blk,
            "b1t": b1_t, "b2t": b2_t, "crep": c_rep, "stats": stats_in,
        })
    res_b = _run(nc_b, in_maps, list(range(NCORES)), "b")
    aux = np.float32(res_b[0]["aux"].reshape(()))
    # y blocks -> (cap, D) scaled contributions per expert
    ye_all = np.stack([
        res_b[e]["y"].transpose(2, 0, 1).reshape(cap, D_MODEL)
        for e in range(N_EXPERTS)
    ])

    # ---- host: route contributions to token owners (gather only)
    tcol, ecol = np.nonzero(nz)          # row-major: token-major pairs
    e1 = ecol[0::2]
    e2 = ecol[1::2]
    posmap = np.zeros((T, N_EXPERTS), np.int64)
    for e in range(N_EXPERTS):
        posmap[ids[e], e] = np.arange(len(ids[e]))
    tok = np.arange(T)
    p1 = posmap[tok, e1]
    p2 = posmap[tok, e2]
    a_all = ye_all[e1, p1]               # (T, D) gather
    b_all = ye_all[e2, p2]

    nc_c = _get("c")
    in_maps_c = [
        {
            "ca": np.ascontiguousarray(
                a_all[r * 1024:(r + 1) * 1024].reshape(8, 128, D_MODEL)
            ),
            "cb": np.ascontiguousarray(
                b_all[r * 1024:(r + 1) * 1024].reshape(8, 128, D_MODEL)
            ),
        }
        for r in range(NCORES)
    ]
    res_c = _run(nc_c, in_maps_c, list(range(NCORES)), "c")
    out = np.concatenate(
        [res_c[r]["co"].reshape(1024, D_MODEL) for r in range(NCORES)]
    ).reshape(B_DIM, S_DIM, D_MODEL)
    return out, aux


# revision 24
# speedup vs baseline: 1.1814x; 1.0515x over previous
# MoE FFN (top-2 of 8 experts) Trainium2 kernel.
#
# Strategy (expert-parallel, per sharding hint):
#   Phase A (8 cores):  router, token-sharded — exact-fp32 logits via PE,
#                       softmax/top-2/combine weights on device; per-shard
#                       aux-loss partial stats.
#   Host:               builds per-expert token gather lists from the device-
#                       computed combine weights (data movement only, no math),
#                       stages gathered/blocked operands per core.
#   Phase B (8 cores):  core e runs expert e's FFN over its gathered tokens:
#                       h = gelu(x @ w1.T + b1); y = (h @ w2.T + b2) * combine.
#                       MM1 in float32r (full-rate fp32), MM2 in bf16.
#                       Also reduces the aux-loss stats to the scalar loss.
#   Host:               routes each token's two expert contributions to its
#                       owner core (gather only).
#   Phase C (8 cores):  out = contribution_a + contribution_b (device add),
#                       token-sharded output.
import numpy as np

D_MODEL, D_FF, N_EXPERTS, TOP_K = 1024, 4096, 8, 2
B_DIM, S_DIM = 4, 2048
T = B_DIM * S_DIM          # 8192 tokens
NCORES = 8
TSH = T // NCORES          # tokens per phase-A shard
DEFAULT_CAP = 2176         # per-expert token capacity (multiple of 128)

_cache = {}
LAST_EXEC_NS = {}


def _run(nc, in_maps, core_ids, label):
    import os
    from concourse import bass_utils
    trace = bool(os.environ.get("MOE_TRACE"))
    res = bass_utils.run_bass_kernel_spmd(
        nc, in_maps, core_ids=core_ids, trace=trace
    )
    if trace:
        LAST_EXEC_NS[label] = res.exec_time_ns
    return res.results


def _mybir():
    import concourse.mybir as mybir
    return mybir


# ----------------------------------------------------------------- phase A
def _build_phase_a():
    import concourse.tile as tile
    from concourse import bacc
    mybir = _mybir()
    F32 = mybir.dt.float32
    AF = mybir.ActivationFunctionType
    OP = mybir.AluOpType
    AX = mybir.AxisListType
    NT = TSH // 128            # 8 token tiles per shard

    nc = bacc.Bacc("TRN2", target_bir_lowering=False, debug=False)
    xTs = nc.dram_tensor("xTs", [8, 128, TSH], F32, kind="ExternalInput")
    gwT = nc.dram_tensor("gwT", [8, 128, 8], F32, kind="ExternalInput")
    eye8 = nc.dram_tensor("eye8", [8, 8], F32, kind="ExternalInput")
    comb_o = nc.dram_tensor("combs", [TSH, 8], F32, kind="ExternalOutput")
    stat_o = nc.dram_tensor("stats", [8, 2], F32, kind="ExternalOutput")

    with tile.TileContext(nc) as tc:
        with (
            tc.tile_pool(name="const", bufs=1) as cpool,
            tc.tile_pool(name="xin", bufs=2) as xpool,
            tc.tile_pool(name="wide", bufs=1) as wpool,
            tc.tile_pool(name="small", bufs=2) as spool,
            tc.tile_pool(name="psA", bufs=2, space="PSUM") as ppa,
            tc.tile_pool(name="psB", bufs=2, space="PSUM") as ppb,
            tc.tile_pool(name="psum_stat", bufs=1, space="PSUM") as pstat,
        ):
            gw_sb = cpool.tile([128, 8, 8], F32)
            nc.sync.dma_start(gw_sb[:], gwT[:].transpose([1, 0, 2]))
            eye_sb = cpool.tile([8, 8], F32)
            nc.sync.dma_start(eye_sb[:], eye8[:])
            ones = cpool.tile([128, 1], F32)
            nc.vector.memset(ones[:], 1.0)

            # logits^T [8, TSH] via PE (exact fp32 — routing decisions must
            # match the fp32 reference at tiny top2/top3 gaps)
            lgT = wpool.tile([8, TSH], F32, tag="lgT")
            for tcn in range(TSH // 512):
                xch = xpool.tile([128, 8, 512], F32, tag="xch")
                nc.sync.dma_start(
                    xch[:],
                    xTs[:, :, tcn * 512:(tcn + 1) * 512].transpose([1, 0, 2]),
                )
                ps_t = ppa.tile([8, 512], F32, tag="pst")
                for di in range(8):
                    nc.tensor.matmul(
                        ps_t[:], gw_sb[:, di, :], xch[:, di, :],
                        start=(di == 0), stop=(di == 7),
                    )
                nc.vector.tensor_copy(lgT[:, tcn * 512:(tcn + 1) * 512], ps_t[:])

            # transpose to [128, NT, 8] (PE transpose via identity)
            lg = wpool.tile([128, NT, 8], F32, tag="lg")
            for tt in range(NT):
                trps = ppb.tile([128, 8], F32, tag="trps")
                nc.tensor.transpose(
                    trps[:], lgT[:, tt * 128:(tt + 1) * 128], eye_sb[:]
                )
                nc.vector.tensor_copy(lg[:, tt, :], trps[:])

            # wide softmax / top-2 over the expert axis (innermost, size 8)
            shp = [128, NT, 8]
            ex = wpool.tile(shp, F32, tag="ex")
            nc.scalar.activation(ex[:], lg[:], AF.Exp)
            s = wpool.tile([128, NT], F32, tag="s")
            nc.vector.tensor_reduce(s[:], ex[:], axis=AX.X, op=OP.add)
            r = wpool.tile([128, NT], F32, tag="r")
            nc.vector.reciprocal(r[:], s[:])
            p = wpool.tile(shp, F32, tag="p")
            nc.vector.tensor_tensor(
                p[:], ex[:], r[:].unsqueeze(2).broadcast_to(shp), OP.mult
            )
            m1 = wpool.tile([128, NT], F32, tag="m1")
            nc.vector.tensor_reduce(m1[:], ex[:], axis=AX.X, op=OP.max)
            mask1 = wpool.tile(shp, F32, tag="mask1")
            nc.vector.tensor_tensor(
                mask1[:], ex[:], m1[:].unsqueeze(2).broadcast_to(shp),
                OP.is_equal,
            )
            ex2 = wpool.tile(shp, F32, tag="ex2")
            nc.vector.scalar_tensor_tensor(
                ex2[:], mask1[:], -100.0, ex[:], op0=OP.mult, op1=OP.add
            )
            m2 = wpool.tile([128, NT], F32, tag="m2")
            nc.vector.tensor_reduce(m2[:], ex2[:], axis=AX.X, op=OP.max)
            mask2 = wpool.tile(shp, F32, tag="mask2")
            nc.vector.tensor_tensor(
                mask2[:], ex2[:], m2[:].unsqueeze(2).broadcast_to(shp),
                OP.is_equal,
            )
            s12 = wpool.tile([128, NT], F32, tag="s12")
            nc.vector.tensor_tensor(s12[:], m1[:], m2[:], OP.add)
            r12 = wpool.tile([128, NT], F32, tag="r12")
            nc.vector.reciprocal(r12[:], s12[:])
            c1 = wpool.tile([128, NT], F32, tag="c1")
            nc.vector.tensor_tensor(c1[:], m1[:], r12[:], OP.mult)
            c2 = wpool.tile([128, NT], F32, tag="c2")
            nc.vector.tensor_tensor(c2[:], m2[:], r12[:], OP.mult)
            t1 = wpool.tile(shp, F32, tag="t1")
            nc.vector.tensor_tensor(
                t1[:], mask1[:], c1[:].unsqueeze(2).broadcast_to(shp), OP.mult
            )
            t2 = wpool.tile(shp, F32, tag="t2")
            nc.vector.tensor_tensor(
                t2[:], mask2[:], c2[:].unsqueeze(2).broadcast_to(shp), OP.mult
            )
            comb = wpool.tile(shp, F32, tag="comb")
            nc.vector.tensor_tensor(comb[:], t1[:], t2[:], OP.add)
            # combine[t, e]  with t = tt*128 + partition
            nc.sync.dma_start(
                comb_o[:].rearrange("(a b) e -> b a e", b=128), comb[:]
            )

            # aux-loss partial stats for this shard: counts, prob sums
            msum = wpool.tile(shp, F32, tag="msum")
            nc.vector.tensor_tensor(msum[:], mask1[:], mask2[:], OP.add)
            cnt_red = spool.tile([128, 8], F32, tag="cnt")
            nc.vector.tensor_reduce(
                cnt_red[:], msum[:].transpose([0, 2, 1]), axis=AX.X, op=OP.add
            )
            p_red = spool.tile([128, 8], F32, tag="pred")
            nc.vector.tensor_reduce(
                p_red[:], p[:].transpose([0, 2, 1]), axis=AX.X, op=OP.add
            )
            ps_cnt = pstat.tile([8, 1], F32, tag="pscnt")
            nc.tensor.matmul(ps_cnt[:], cnt_red[:], ones[:], start=True, stop=True)
            ps_p = pstat.tile([8, 1], F32, tag="psp")
            nc.tensor.matmul(ps_p[:], p_red[:], ones[:], start=True, stop=True)
            stat_sb = spool.tile([8, 2], F32, tag="statsb")
            nc.vector.tensor_copy(stat_sb[:, 0:1], ps_cnt[:])
            nc.vector.tensor_copy(stat_sb[:, 1:2], ps_p[:])
            nc.sync.dma_start(stat_o[:], stat_sb[:])

    nc.compile()
    return nc


def _build_phase_b_aux(nc, tc, tile, mybir, stats, aux_o, spool):
    # DVE-only on a single partition; pool stays open for the whole kernel
    # (early-closed pools corrupt later allocations).
    # stats layout: [1, kind(2), expert(8), core(8)] on partition 0.
    F32 = mybir.dt.float32
    OP = mybir.AluOpType
    AX = mybir.AxisListType
    st_sb = spool.tile([1, 2, 8, 8], F32, tag="stsb")
    nc.gpsimd.dma_start(st_sb[:], stats[:])
    red = spool.tile([1, 2, 8], F32, tag="red")
    nc.vector.tensor_reduce(red[:], st_sb[:], axis=AX.X, op=OP.add)
    terms = spool.tile([1, 8], F32, tag="terms")
    nc.vector.tensor_tensor(terms[:], red[:, 0, :], red[:, 1, :], OP.mult)
    tot = spool.tile([1, 1], F32, tag="tot")
    nc.vector.tensor_reduce(tot[:], terms[:], axis=AX.X, op=OP.add)
    aux_sb = spool.tile([1, 1], F32, tag="auxsb")
    scale = float(N_EXPERTS) / (float(T) * TOP_K * float(T))
    nc.scalar.mul(aux_sb[:], tot[:], scale)
    nc.gpsimd.dma_start(aux_o[:], aux_sb[:])


# ----------------------------------------------------------------- phase B
def _build_phase_b(cap):
    import concourse.tile as tile
    from concourse import bacc
    mybir = _mybir()
    F32 = mybir.dt.float32
    F32R = mybir.dt.float32r
    BF16 = mybir.dt.bfloat16
    AF = mybir.ActivationFunctionType
    OP = mybir.AluOpType

    assert cap % 128 == 0
    # token groups of <=1280 (h for one group stays SBUF-resident);
    # each group is cut into matmul chunks of 256..512 columns (float32r
    # runs quarter-rate below 256).
    def cut(gs):
        sizes = []
        left = gs
        while left > 512 + 256:
            sizes.append(512)
            left -= 512
        if left > 512:
            a = -(-left // 2 // 64) * 64
            sizes += [a, left - a]
        else:
            sizes.append(left)
        assert sum(sizes) == gs and all(256 <= nk <= 512 for nk in sizes)
        out, o = [], 0
        for nk in sizes:
            out.append((o, nk))
            o += nk
        return out

    groups = []
    g0 = 0
    ngroups = -(-cap // 1280)
    gsize = -(-cap // ngroups // 128) * 128
    while g0 < cap:
        gs = min(gsize, cap - g0)
        groups.append((g0, gs, cut(gs)))
        g0 += gs

    nc = bacc.Bacc("TRN2", target_bir_lowering=False, debug=False)
    xg = nc.dram_tensor("xg", [8, 128, cap], F32R, kind="ExternalInput")
    w1 = nc.dram_tensor("w1b", [32, 8, 128, 128], F32R, kind="ExternalInput")
    w2 = nc.dram_tensor("w2b", [8, 128, 32, 128], F32, kind="ExternalInput")
    b1 = nc.dram_tensor("b1t", [128, 32], F32, kind="ExternalInput")
    b2 = nc.dram_tensor("b2t", [128, 8], F32, kind="ExternalInput")
    crep = nc.dram_tensor("crep", [128, cap], F32, kind="ExternalInput")
    stats = nc.dram_tensor("stats", [1, 2, 8, 8], F32, kind="ExternalInput")
    y = nc.dram_tensor("y", [8, 128, cap], F32, kind="ExternalOutput")
    aux_o = nc.dram_tensor("aux", [1, 1], F32, kind="ExternalOutput")

    with tile.TileContext(nc) as tc:
        with (
            tc.tile_pool(name="const", bufs=1) as cpool,
            tc.tile_pool(name="small", bufs=1) as spool,
            tc.tile_pool(name="dram", bufs=1, space="DRAM") as dpool,
        ):
            b1_sb = cpool.tile([128, 32], F32)
            nc.sync.dma_start(b1_sb[:], b1[:])
            b2_sb = cpool.tile([128, 8], F32)
            nc.sync.dma_start(b2_sb[:], b2[:])
            c_sb = cpool.tile([128, cap], F32)
            nc.gpsimd.dma_start(c_sb[:], crep[:])

            # aux loss from the phase-A partial stats (tiny, replicated)
            _build_phase_b_aux(nc, tc, tile, mybir, stats, aux_o, spool)
            # w2 -> bf16 cast (DRAM->DRAM via SBUF), interleaved into the
            # first group's MM1 so its DMA traffic stays off the critical
            # path of the first xg/w1 loads.
            w2bf_d = dpool.tile([8, 128, 32, 128], BF16)

            def w2_cast_step(step):
                di, hf = step // 2, (step % 2) * 16
                stg = stgpool.tile([128, 16, 128], F32, tag="stg")
                nc.gpsimd.dma_start(stg[:], w2[di, :, hf:hf + 16, :])
                w2c = w2cpool.tile([128, 16, 128], BF16, tag="w2c")
                nc.vector.tensor_copy(w2c[:], stg[:])
                nc.gpsimd.dma_start(w2bf_d[di, :, hf:hf + 16, :], w2c[:])

            with (
                tc.tile_pool(name="w2stg", bufs=1) as stgpool,
                tc.tile_pool(name="w2c", bufs=1) as w2cpool,
                tc.tile_pool(name="xg", bufs=1) as xpool,
                tc.tile_pool(name="h", bufs=1) as hpool,
                tc.tile_pool(name="w1", bufs=3) as w1pool,
                tc.tile_pool(name="w2s", bufs=2) as w2spool,
                tc.tile_pool(name="yout", bufs=3) as ypool,
                tc.tile_pool(name="psum", bufs=6, space="PSUM") as pp,
            ):
                for gi, (g0, gs, chunks) in enumerate(groups):
                    xgs = xpool.tile([128, 8, gs], F32R, tag="xgs")

                    def xg_load(ci):
                        o, nk = chunks[ci]
                        nc.sync.dma_start(
                            xgs[:, :, o:o + nk],
                            xg[:, :, g0 + o:g0 + o + nk].transpose([1, 0, 2]),
                        )
                    xg_load(0)
                    h_sb = hpool.tile([128, 32, gs], BF16, tag="h")
                    # ---- MM1: h = gelu(x @ w1.T + b1)  (float32r)
                    for fi in range(32):
                        w1t = w1pool.tile([128, 8, 128], F32R, tag="w1t")
                        nc.sync.dma_start(w1t[:], w1[fi].transpose([1, 0, 2]))
                        if fi == 0:
                            for _ci in range(1, len(chunks)):
                                xg_load(_ci)
                        pss = []
                        for ci, (o, nk) in enumerate(chunks):
                            pst = pp.tile([128, 512], F32, tag="ps",
                                          name=f"ps{ci}")
                            pss.append(pst)
                        for di in range(8):
                            for ci, (o, nk) in enumerate(chunks):
                                nc.tensor.matmul(
                                    pss[ci][:, 0:nk],
                                    w1t[:, di, :],
                                    xgs[:, di, o:o + nk],
                                    start=(di == 0),
                                    stop=(di == 7),
                                )
                        for ci, (o, nk) in enumerate(chunks):
                            nc.scalar.activation(
                                h_sb[:, fi, o:o + nk], pss[ci][:, 0:nk],
                                AF.Gelu, bias=b1_sb[:, fi:fi + 1],
                            )
                        if gi == 0 and fi % 2 == 1:
                            w2_cast_step(fi // 2)
                    # ---- MM2: y = (h @ w2.T + b2) * combine   (bf16)
                    for di in range(8):
                        w2t = w2spool.tile([128, 32, 128], BF16, tag="w2t")
                        nc.sync.dma_start(w2t[:], w2bf_d[di])
                        for (o, nk) in chunks:
                            psy = pp.tile([128, 512], F32, tag="ps")
                            for fj in range(32):
                                nc.tensor.matmul(
                                    psy[:, 0:nk],
                                    w2t[:, fj, :],
                                    h_sb[:, fj, o:o + nk],
                                    start=(fj == 0),
                                    stop=(fj == 31),
                                )
                            yt = ypool.tile([128, 512], F32, tag="yt")
                            nc.vector.scalar_tensor_tensor(
                                yt[:, 0:nk], psy[:, 0:nk], b2_sb[:, di:di + 1],
                                c_sb[:, g0 + o:g0 + o + nk],
                                op0=OP.add, op1=OP.mult,
                            )
                            nc.sync.dma_start(
                                y[di, :, g0 + o:g0 + o + nk], yt[:, 0:nk]
                            )

    nc.compile()
    return nc


# ----------------------------------------------------------------- phase C
def _build_phase_c():
    import concourse.tile as tile
    from concourse import bacc
    mybir = _mybir()
    F32 = mybir.dt.float32
    OP = mybir.AluOpType

    nc = bacc.Bacc("TRN2", target_bir_lowering=False, debug=False)
    a_in = nc.dram_tensor("ca", [8, 128, 1024], F32, kind="ExternalInput")
    b_in = nc.dram_tensor("cb", [8, 128, 1024], F32, kind="ExternalInput")
    o_out = nc.dram_tensor("co", [8, 128, 1024], F32, kind="ExternalOutput")

    with tile.TileContext(nc) as tc:
        with (
            tc.tile_pool(name="io", bufs=2) as iop,
        ):
            for half in range(2):
                sl = slice(half * 4, half * 4 + 4)
                at = iop.tile([128, 4, 1024], F32, tag="a")
                nc.sync.dma_start(at[:], a_in[sl].transpose([1, 0, 2]))
                bt = iop.tile([128, 4, 1024], F32, tag="b")
                nc.sync.dma_start(bt[:], b_in[sl].transpose([1, 0, 2]))
                ot = iop.tile([128, 4, 1024], F32, tag="o")
                nc.vector.tensor_tensor(ot[:], at[:], bt[:], OP.add)
                nc.sync.dma_start(o_out[sl].transpose([1, 0, 2]), ot[:])

    nc.compile()
    return nc


def _get(phase, *args):
    key = (phase,) + args
    if key not in _cache:
        if phase == "a":
            _cache[key] = _build_phase_a()
        elif phase == "b":
            _cache[key] = _build_phase_b(*args)
        else:
            _cache[key] = _build_phase_c()
    return _cache[key]


# ----------------------------------------------------------------- driver
def kernel(x, gate_w, w1, b1, w2, b2):
    x = np.ascontiguousarray(np.asarray(x, np.float32))
    gate_w = np.ascontiguousarray(np.asarray(gate_w, np.float32))
    w1 = np.ascontiguousarray(np.asarray(w1, np.float32))
    b1 = np.ascontiguousarray(np.asarray(b1, np.float32))
    w2 = np.ascontiguousarray(np.asarray(w2, np.float32))
    b2 = np.ascontiguousarray(np.asarray(b2, np.float32))

    x_flat = x.reshape(T, D_MODEL)

    # ---- phase A: routing on device (8 cores, token-sharded)
    xT = x_flat.T.reshape(8, 128, T)
    gwT = np.ascontiguousarray(gate_w.T.reshape(8, 128, 8))
    eye8 = np.eye(8, dtype=np.float32)
    nc_a = _get("a")
    in_maps_a = [
        {
            "xTs": np.ascontiguousarray(xT[:, :, r * TSH:(r + 1) * TSH]),
            "gwT": gwT,
            "eye8": eye8,
        }
        for r in range(NCORES)
    ]
    res_a = _run(nc_a, in_maps_a, list(range(NCORES)), "a")
    combine = np.concatenate([res_a[r]["combs"] for r in range(NCORES)])
    stats_v = np.stack([res_a[r]["stats"] for r in range(NCORES)])  # (core,e,kind)
    stats_in = np.ascontiguousarray(
        stats_v.transpose(2, 1, 0)[None].astype(np.float32)  # (1,kind,e,core)
    )

    # ---- host: build gather lists (indexing only)
    nz = combine > 0
    ids = [np.nonzero(nz[:, e])[0] for e in range(N_EXPERTS)]
    maxcnt = max(len(i) for i in ids)
    cap = DEFAULT_CAP
    while cap < maxcnt:
        cap += 256

    nc_b = _get("b", cap)
    in_maps = []
    for e in range(N_EXPERTS):
        n = len(ids[e])
        xe = np.zeros((cap, D_MODEL), np.float32)
        xe[:n] = x_flat[ids[e]]
        xg_blk = np.ascontiguousarray(xe.T.reshape(8, 128, cap))
        w1_blk = np.ascontiguousarray(
            w1[e].reshape(32, 128, 8, 128).transpose(0, 2, 3, 1)
        )
        w2_blk = np.ascontiguousarray(
            w2[e].reshape(8, 128, 32, 128).transpose(0, 3, 2, 1)
        )
        b1_t = np.ascontiguousarray(b1[e].reshape(32, 128).T)
        b2_t = np.ascontiguousarray(b2[e].reshape(8, 128).T)
        ce = np.zeros((cap,), np.float32)
        ce[:n] = combine[ids[e], e]
        c_rep = np.ascontiguousarray(np.broadcast_to(ce, (128, cap)))
        in_maps.append({
            "xg": xg_blk, "w1b": w1_blk, "w2b": w2_blk,
            "b1t": b1_t, "b2t": b2_t, "crep": c_rep, "stats": stats_in,
        })
    res_b = _run(nc_b, in_maps, list(range(NCORES)), "b")
    aux = np.float32(res_b[0]["aux"].reshape(()))
    # y blocks -> (cap, D) scaled contributions per expert
    ye_all = np.stack([
        res_b[e]["y"].transpose(2, 0, 1).reshape(cap, D_MODEL)
        for e in range(N_EXPERTS)
    ])

    # ---- host: route contributions to token owners (gather only)
    tcol, ecol = np.nonzero(nz)          # row-major: token-major pairs
    e1 = ecol[0::2]
    e2 = ecol[1::2]
    posmap = np.zeros((T, N_EXPERTS), np.int64)
    for e in range(N_EXPERTS):
        posmap[ids[e], e] = np.arange(len(ids[e]))
    tok = np.arange(T)
    p1 = posmap[tok, e1]
    p2 = posmap[tok, e2]
    a_all = ye_all[e1, p1]               # (T, D) gather
    b_all = ye_all[e2, p2]

    nc_c = _get("c")
    in_maps_c = [
        {
            "ca": np.ascontiguousarray(
                a_all[r * 1024:(r + 1) * 1024].reshape(8, 128, D_MODEL)
            ),
            "cb": np.ascontiguousarray(
                b_all[r * 1024:(r + 1) * 1024].reshape(8, 128, D_MODEL)
            ),
        }
        for r in range(NCORES)
    ]
    res_c = _run(nc_c, in_maps_c, list(range(NCORES)), "c")
    out = np.concatenate(
        [res_c[r]["co"].reshape(1024, D_MODEL) for r in range(NCORES)]
    ).reshape(B_DIM, S_DIM, D_MODEL)
    return out, aux
